# revision 11
# baseline (speedup 1.0000x reference)
"""Trainium2 Bass kernel for the GBottleneck GCN (8-core SPMD).

Strategy (graph/data parallel, per the sharding hint):
  - Nodes are dealt to 8 cores so that every core has an IDENTICAL padded
    "degree class" profile (required: SPMD shares one instruction stream).
  - State is kept transposed in SBUF: x_T [128 feat, S_pad nodes] f32.
  - Per GCN layer: xw = W^T @ x_T on TensorE; y = xw * norm (DVE);
    y is PE-transposed to node-major fp16 rows and DMA'd to a DRAM shard;
    an AllGather replicates the full fp16 node table to every core;
    per-edge messages are fetched with transpose-mode dma_gather
    (output layout [128 feat, edge cols]) and segment-summed with
    3D-AP tensor_reduce runs (edges pre-sorted by destination, constant
    padded degree per run); merge = (agg0+agg1+y)*norm + b, leaky relu.
  - Table indices are int16 (dma_gather), so the node table is split in two
    halves (cores 0-4 / 5-7); each half's rows < 32768. idx -1 gathers the
    shard's zero row (padding contributes exactly 0).

kernel(**inputs) takes FULL inputs and returns (x_out [N,64], x [N,128]).
"""
import sys
import os

sys.path.insert(0, "/opt/trn_rl_repo")

import numpy as np

# ---------------------------------------------------------------------------
# Problem constants (hardcoded; kernel.py must be self-contained)
# ---------------------------------------------------------------------------
N_NODES = 50000
N_EDGES = 800000
DIM_IN = 128
DIM_H = 128
DIM_OUT = 64
N_BLOCKS = 6
NEG_SLOPE = 0.01
N_CORES = 8
N_HALF0 = 5          # cores 0..4 feed table half 0
COARSE = 2           # degree padding granularity
CH = 4096            # gather chunk columns (multiple of 128)
P = 128
MM_BLK = 512


# ---------------------------------------------------------------------------
# Host-side graph preprocessing
# ---------------------------------------------------------------------------
def build_plan(edges, n_nodes, n_cores=N_CORES, n_half0=N_HALF0, coarse=COARSE,
               ch=CH):
    src = edges[0].astype(np.int64)
    dst = edges[1].astype(np.int64)
    deg = 1.0 + np.bincount(dst, minlength=n_nodes).astype(np.float64)
    norm = (1.0 / np.sqrt(deg)).astype(np.float32)

    # phase 1: deal nodes round-robin in degree-sorted order (edge balance)
    order = np.argsort(-deg, kind="stable")
    core_of = np.empty(n_nodes, np.int64)
    core_of[order] = np.arange(n_nodes) % n_cores
    half_of = (core_of >= n_half0).astype(np.int64)

    sh = half_of[src]
    d0 = np.bincount(dst[sh == 0], minlength=n_nodes)
    d1 = np.bincount(dst[sh == 1], minlength=n_nodes)
    c = coarse
    pd0 = np.where(d0 > 0, ((d0 + c - 1) // c) * c, 0)
    pd1 = np.where(d1 > 0, ((d1 + c - 1) // c) * c, 0)

    # phase 2: re-deal within halves, class by class (keeps half_of intact),
    # padding every core to the same per-class count.
    cls_key = pd0 * 10000 + pd1
    uniq = np.unique(cls_key)
    upd0, upd1 = uniq // 10000, uniq % 10000
    cls_order = np.lexsort((upd1, upd0))

    nodes_by_cls_half = {}
    for k in cls_order:
        for h in (0, 1):
            nodes_by_cls_half[(k, h)] = np.where(
                (cls_key == uniq[k]) & (half_of == h))[0]

    core_nodes = [[] for _ in range(n_cores)]
    pos_pd = []  # (pd0, pd1) per position, shared across cores
    for k in cls_order:
        for h, cores_h in ((0, list(range(n_half0))),
                           (1, list(range(n_half0, n_cores)))):
            nodes_kh = nodes_by_cls_half[(k, h)]
            nh = len(cores_h)
            cnt = -(-len(nodes_kh) // nh) if len(nodes_kh) else 0
            for j, core in enumerate(cores_h):
                mine = nodes_kh[j::nh]
                core_nodes[core].extend(mine.tolist())
                core_nodes[core].extend([-1] * (cnt - len(mine)))
        mx = max(len(cn) for cn in core_nodes)
        for cn in core_nodes:
            cn.extend([-1] * (mx - len(cn)))
        pos_pd.extend([(int(upd0[k]), int(upd1[k]))] * (mx - len(pos_pd)))

    s_real = len(core_nodes[0])
    s_pad = ((s_real + 127) // 128) * 128
    for cn in core_nodes:
        cn.extend([-1] * (s_pad - s_real))
    pos_pd.extend([(0, 0)] * (s_pad - s_real))
    core_nodes = [np.array(cn, np.int64) for cn in core_nodes]
    pos_pd0 = np.array([p[0] for p in pos_pd], np.int64)
    pos_pd1 = np.array([p[1] for p in pos_pd], np.int64)
    kb = s_pad // 128
    shard_rows = s_pad + 1
    assert n_half0 * shard_rows < 32768, (n_half0, shard_rows)

    # local position + final core of each real node (phase 2 re-deals cores!)
    local_pos = np.full(n_nodes, -1, np.int64)
    core_of2 = np.full(n_nodes, -1, np.int64)
    for cc in range(n_cores):
        cn = core_nodes[cc]
        real = cn >= 0
        local_pos[cn[real]] = np.nonzero(real)[0]
        core_of2[cn[real]] = cc
    assert (local_pos >= 0).all()
    assert ((core_of2 >= n_half0).astype(np.int64) == half_of).all()

    # table row of each node (within its half's gather window)
    tbl_row = (core_of2 * shard_rows + 1
               + (local_pos % 128) * kb + (local_pos // 128))
    half1_base = n_half0 * shard_rows
    rel_row = np.where(half_of == 1, tbl_row - half1_base, tbl_row)
    assert rel_row.max() < 32768

    # per (core, half): concatenated padded idx columns, position order.
    # Edge lists grouped by destination; order within a node arbitrary.
    edge_order = np.lexsort((src, dst))
    s_srt, d_srt = src[edge_order], dst[edge_order]
    h_srt = half_of[s_srt]
    # start offset of each dst's edges in (dst, half)-sorted order:
    order2 = np.lexsort((h_srt, np.zeros_like(d_srt)))  # stable, keep d sort
    # simpler: fully sort by (dst, half(src))
    eo = np.lexsort((half_of[src], dst))
    s_srt, d_srt = src[eo], dst[eo]
    h_srt = half_of[s_srt]
    # per dst: count of half0/half1 edges and their slice starts
    starts = np.searchsorted(d_srt, np.arange(n_nodes))
    ends = np.searchsorted(d_srt, np.arange(n_nodes), side="right")

    # chunking structure shared across cores (positions with pd>0)
    def build_chunks(pos_d):
        chunks = []   # list of dicts: positions [(pos, d, col_off)], cols_used, runs
        cur = {"items": [], "cols": 0}
        for pos in range(s_pad):
            d = int(pos_d[pos])
            if d == 0:
                continue
            if cur["cols"] + d > ch:
                chunks.append(cur)
                cur = {"items": [], "cols": 0}
            cur["items"].append((pos, d, cur["cols"]))
            cur["cols"] += d
        if cur["items"]:
            chunks.append(cur)
        # runs per chunk: maximal constant-d, consecutive-pos ranges
        for cidx, chk in enumerate(chunks):
            runs = []
            items = chk["items"]
            i = 0
            while i < len(items):
                pos0, d, col0 = items[i]
                j = i
                while (j + 1 < len(items)
                       and items[j + 1][1] == d
                       and items[j + 1][0] == items[j][0] + 1):
                    j += 1
                runs.append((pos0, j - i + 1, d, col0))
                i = j + 1
            chk["runs"] = runs
        # zero runs (pd == 0) for memset
        zruns = []
        pos = 0
        while pos < s_pad:
            if pos_d[pos] == 0:
                q = pos
                while q < s_pad and pos_d[q] == 0:
                    q += 1
                zruns.append((pos, q - pos))
                pos = q
            else:
                pos += 1
        return chunks, zruns

    chunks0, zruns0 = build_chunks(pos_pd0)
    chunks1, zruns1 = build_chunks(pos_pd1)

    # idx arrays per (core, half): [128, n_chunks*ch/16] int16 wrapped
    def build_idx(cc, half, chunks, pos_d):
        cn = core_nodes[cc]
        n_chunks = len(chunks)
        flat = np.full(n_chunks * ch, -1, np.int16)
        nreal = np.zeros(n_chunks, np.int64)
        for cidx, chk in enumerate(chunks):
            base = cidx * ch
            for (pos, d, col0) in chk["items"]:
                g = cn[pos]
                if g < 0:
                    continue
                st = starts[g]
                en = ends[g]
                if half == 0:
                    lo, hi = st, st + np.searchsorted(h_srt[st:en], 1)
                else:
                    lo, hi = st + np.searchsorted(h_srt[st:en], 1), en
                cnt = hi - lo
                assert cnt <= d
                vals = rel_row[s_srt[lo:hi]].astype(np.int16)
                flat[base + col0: base + col0 + cnt] = vals
                nreal[cidx] += cnt
        wrapped = np.zeros((128, n_chunks * ch // 16), np.int16)
        cols = flat.reshape(-1, 16)  # [ncols, 16]
        w16 = cols.T  # [16, ncols]
        for rep in range(8):
            wrapped[rep * 16:(rep + 1) * 16] = w16
        return wrapped, nreal

    idx_arrays = {}
    nreal_arrays = {}
    for half, chunks, pos_d in ((0, chunks0, pos_pd0), (1, chunks1, pos_pd1)):
        nr = None
        for cc in range(n_cores):
            w, nreal = build_idx(cc, half, chunks, pos_d)
            idx_arrays[(cc, half)] = w
            if nr is None:
                nr = nreal
        # num_idxs_reg must match per-core counts; they differ per core!
        # store per-core counts
        for cc in range(n_cores):
            pass
    # recompute per-core nreal (needed per-core? instruction uses a constant ->
    # must be IDENTICAL across cores). Use num_idxs_reg = position of last
    # real +1 is what matters for perf; correctness needs exact count.
    # -> use per-chunk count of the TEMPLATE... see note in emit: we pass
    # num_idxs_reg equal to CH and pad with real zero-row indices instead
    # of -1 so the count is identical across cores.
    return dict(
        norm=norm, core_of=core_of2, half_of=half_of, core_nodes=core_nodes,
        local_pos=local_pos, s_pad=s_pad, kb=kb, shard_rows=shard_rows,
        half1_base=half1_base, chunks0=chunks0, chunks1=chunks1,
        zruns0=zruns0, zruns1=zruns1, idx_arrays=idx_arrays,
        pos_pd0=pos_pd0, pos_pd1=pos_pd1, n_cores=n_cores, n_half0=n_half0,
        ch=ch,
    )


def _fix_idx_counts(plan):
    """Replace -1 padding with 0 (the zero row) so every chunk has the same
    non-negative count (CH) on every core -> num_idxs_reg constant = CH.
    Index 0 points at a shard's zero row, gathering exact zeros."""
    for key, w in plan["idx_arrays"].items():
        w[w < 0] = 0
    return plan


# ---------------------------------------------------------------------------
# Per-core input construction
# ---------------------------------------------------------------------------
def build_in_maps(plan, inputs, weights_list, biases_list):
    """inputs: [N, IN] f32; weights_list: L arrays [128,128] (lhsT layout,
    [in, out], out-padded to 128); biases_list: L arrays [128] (padded)."""
    n_cores = plan["n_cores"]
    s_pad = plan["s_pad"]
    L = len(weights_list)
    w_host = np.stack(weights_list, axis=1).astype(np.float32)  # [128, L, 128]
    w_host = np.ascontiguousarray(w_host.reshape(128, L * 128))
    b_host = np.stack(biases_list, axis=1).astype(np.float32)   # [128, L]

    in_maps = []
    for cc in range(n_cores):
        cn = plan["core_nodes"][cc]
        x0 = np.zeros((s_pad, inputs.shape[1]), np.float32)
        nb = np.ones((s_pad,), np.float32)
        real = cn >= 0
        x0[real] = inputs[cn[real]]
        nb[real] = plan["norm"][cn[real]]
        x0_t = np.ascontiguousarray(x0.T)                        # [128, s_pad]
        norm_b = np.ascontiguousarray(
            np.tile(nb[None, :], (128, 1)))                      # [128, s_pad]
        m = {
            "x0": x0_t,
            "normb": norm_b,
            "wts": w_host,
            "bias": b_host,
            "idx0": plan["idx_arrays"][(cc, 0)],
            "idx1": plan["idx_arrays"][(cc, 1)],
        }
        in_maps.append(m)
    return in_maps


def unshard_outputs(plan, results, dim_out=DIM_OUT, dim_h=DIM_H):
    n_nodes = len(plan["core_of"])
    x_full = np.zeros((n_nodes, dim_h), np.float32)
    xo_full = np.zeros((n_nodes, dim_out), np.float32)
    for cc in range(plan["n_cores"]):
        cn = plan["core_nodes"][cc]
        real = cn >= 0
        xt = results[cc]["x_t"]         # [128, s_pad]
        xot = results[cc]["x_out_t"]    # [128, s_pad]
        x_full[cn[real]] = xt.T[real][:, :dim_h]
        xo_full[cn[real]] = xot.T[real][:, :dim_out]
    return xo_full, x_full


# ---------------------------------------------------------------------------
# IR emission
# ---------------------------------------------------------------------------
def emit_kernel(tc, io, plan, n_layers):
    from contextlib import ExitStack
    import concourse.bass as bass
    import concourse.mybir as mybir
    from concourse.masks import make_identity

    ctx = ExitStack()
    nc = tc.nc
    s_pad = plan["s_pad"]
    kb = plan["kb"]
    shard_rows = plan["shard_rows"]
    n_cores = plan["n_cores"]
    n_half0 = plan["n_half0"]
    ch = plan["ch"]
    f32 = mybir.dt.float32
    f16 = mybir.dt.float16
    AL = mybir.AluOpType
    L = n_layers

    n_ch0 = len(plan["chunks0"])
    n_ch1 = len(plan["chunks1"])

    sb = ctx.enter_context(tc.tile_pool(name="state", bufs=1))
    gp = ctx.enter_context(tc.tile_pool(name="gath", bufs=2))
    ps = ctx.enter_context(tc.tile_pool(name="psmm", bufs=2, space="PSUM"))
    pt = ctx.enter_context(tc.tile_pool(name="pstr", bufs=2, space="PSUM"))
    dr = ctx.enter_context(tc.tile_pool(name="dram", bufs=1, space="DRAM"))

    A = sb.tile([P, s_pad], f32, name="A")
    Bt = sb.tile([P, s_pad], f32, name="Bt")
    NB = sb.tile([P, s_pad], f32, name="NB")
    agg0 = sb.tile([P, s_pad], f32, name="agg0")
    agg1 = sb.tile([P, s_pad], f32, name="agg1")
    stage = sb.tile([P, kb, P], f16, name="stage")
    WT = sb.tile([P, L * P], f32, name="WT")
    BT = sb.tile([P, L], f32, name="BT")
    I0 = sb.tile([P, n_ch0 * ch // 16], mybir.dt.int16, name="I0")
    I1 = sb.tile([P, n_ch1 * ch // 16], mybir.dt.int16, name="I1")
    ident = sb.tile([P, P], f32, name="ident")
    z16 = sb.tile([1, P], f16, name="z16")

    shard_b = dr.tile([shard_rows, P], f16, name="shard_b")
    table_b = dr.tile([n_cores * shard_rows, P], f16, name="table_b")

    # ---- setup ----
    make_identity(nc, ident[:])
    nc.sync.dma_start(A[:], io["x0"][:])
    nc.sync.dma_start(NB[:], io["normb"][:])
    nc.sync.dma_start(WT[:], io["wts"][:])
    nc.sync.dma_start(BT[:], io["bias"][:])
    nc.sync.dma_start(I0[:], io["idx0"][:])
    nc.sync.dma_start(I1[:], io["idx1"][:])
    nc.vector.memset(z16[:], 0.0)
    nc.sync.dma_start(shard_b[0:1, :], z16[:])

    shard_rows_view = shard_b[1:, :].rearrange("(p k) f -> p (k f)", p=P)
    table0 = table_b[0:n_half0 * shard_rows, :]
    table1 = table_b[n_half0 * shard_rows:, :]

    def emit_layer(l, S_in, S_out, leaky):
        wl = WT[:, l * P:(l + 1) * P]
        # matmul + y = xw * norm
        off = 0
        while off < s_pad:
            bw = min(MM_BLK, s_pad - off)
            pm = ps.tile([P, MM_BLK], f32, name=f"pm{l}", tag="mm")
            nc.tensor.matmul(out=pm[:, :bw], lhsT=wl, rhs=S_in[:, off:off + bw],
                             start=True, stop=True)
            nc.vector.tensor_tensor(out=S_out[:, off:off + bw],
                                    in0=pm[:, :bw], in1=NB[:, off:off + bw],
                                    op=AL.mult)
            off += bw
        # transpose y -> node-major fp16 stage
        for k in range(kb):
            pk = pt.tile([P, P], f32, name=f"pt{l}", tag="tr")
            nc.tensor.transpose(out=pk[:], in_=S_out[:, k * P:(k + 1) * P],
                                identity=ident[:])
            nc.scalar.copy(out=stage[:, k, :], in_=pk[:])
        nc.sync.dma_start(shard_rows_view, stage[:])
        # replicate table
        nc.gpsimd.collective_compute(
            "AllGather", AL.bypass,
            ins=[shard_b[:].opt()], outs=[table_b[:].opt()],
            replica_groups=[list(range(n_cores))],
        )
        # gather + segmented reduce
        for half, chunks, zruns, itile, tap in (
                (0, plan["chunks0"], plan["zruns0"], I0, table0),
                (1, plan["chunks1"], plan["zruns1"], I1, table1)):
            agg = agg0 if half == 0 else agg1
            for (zoff, zn) in zruns:
                nc.vector.memset(agg[:, zoff:zoff + zn], 0.0)
            for cidx, chk in enumerate(chunks):
                g = gp.tile([P, 1, ch], f16, name=f"g{l}_{half}_{cidx}",
                            tag="g")
                nc.gpsimd.dma_gather(
                    out_ap=g[:], in_ap=tap,
                    idxs_ap=itile[:, cidx * ch // 16:(cidx + 1) * ch // 16],
                    num_idxs=ch, num_idxs_reg=ch, elem_size=P,
                    transpose=True, single_packet=False,
                )
                for (pos0, n, d, col0) in chk["runs"]:
                    g3 = g[:, 0, col0:col0 + n * d].rearrange(
                        "p (n d) -> p n d", d=d)
                    nc.vector.tensor_reduce(
                        out=agg[:, pos0:pos0 + n], in_=g3,
                        axis=mybir.AxisListType.X, op=AL.add)
        # merge: S_out = lrelu((agg0 + agg1 + y) * norm + b)
        nc.vector.tensor_tensor(out=agg0[:], in0=agg0[:], in1=agg1[:],
                                op=AL.add)
        nc.vector.tensor_tensor(out=agg0[:], in0=agg0[:], in1=S_out[:],
                                op=AL.add)
        nc.vector.tensor_tensor(out=agg0[:], in0=agg0[:], in1=NB[:],
                                op=AL.mult)
        if leaky:
            nc.scalar.activation(out=agg1[:], in_=agg0[:],
                                 func=mybir.ActivationFunctionType.Identity,
                                 bias=BT[:, l:l + 1], scale=1.0)
            nc.vector.tensor_scalar_mul(agg0[:], agg1[:], NEG_SLOPE)
            nc.vector.tensor_tensor(out=S_out[:], in0=agg1[:], in1=agg0[:],
                                    op=AL.max)
        else:
            nc.scalar.activation(out=S_out[:], in_=agg0[:],
                                 func=mybir.ActivationFunctionType.Identity,
                                 bias=BT[:, l:l + 1], scale=1.0)

    # layer schedule
    emit_layer(0, A, A, True)
    li = 1
    for i in range((L - 2) // 2):
        emit_layer(li, A, Bt, True)
        li += 1
        emit_layer(li, Bt, Bt, True)
        li += 1
        nc.vector.tensor_tensor(out=A[:], in0=A[:], in1=Bt[:],
                                op=mybir.AluOpType.add)
        nc.vector.tensor_scalar_mul(A[:], A[:], 0.5)
    emit_layer(L - 1, A, Bt, False)

    nc.sync.dma_start(io["x_t"][:], A[:])
    nc.sync.dma_start(io["x_out_t"][:], Bt[:])
    ctx.close()


# ---------------------------------------------------------------------------
# Reference math on host (for mini tests): numpy GCN with plan's layout
# ---------------------------------------------------------------------------
def reference_numpy(inputs, edges, w_in, b_in, w1, b1, w2, b2, w_out, b_out,
                    n_blocks):
    n = inputs.shape[0]
    src, dst = edges[0].astype(np.int64), edges[1].astype(np.int64)
    deg = 1.0 + np.bincount(dst, minlength=n).astype(np.float64)
    norm = (1.0 / np.sqrt(deg)).astype(np.float32)

    def leaky(v):
        return np.where(v >= 0, v, NEG_SLOPE * v)

    def conv(x, W, b):
        xw = x @ W
        msg = xw[src] * norm[src, None]
        agg = np.zeros_like(xw)
        np.add.at(agg, dst, msg)
        agg = agg * norm[:, None] + xw * (norm * norm)[:, None]
        return agg + b

    x = leaky(conv(inputs, w_in, b_in))
    for i in range(n_blocks):
        h = leaky(conv(x, w1[i], b1[i]))
        h = leaky(conv(h, w2[i], b2[i]))
        x = (x + h) * 0.5
    x_out = conv(x, w_out, b_out)
    return x_out, x


# ---------------------------------------------------------------------------
# Weight/bias packing
# ---------------------------------------------------------------------------
def pack_weights(w_in, b_in, w1, b1, w2, b2, w_out, b_out, n_blocks):
    weights, biases = [], []

    def padw(w):
        out = np.zeros((P, P), np.float32)
        out[:w.shape[0], :w.shape[1]] = w
        return out

    def padb(b):
        out = np.zeros((P,), np.float32)
        out[:b.shape[0]] = b
        return out

    weights.append(padw(w_in)); biases.append(padb(b_in))
    for i in range(n_blocks):
        weights.append(padw(w1[i])); biases.append(padb(b1[i]))
        weights.append(padw(w2[i])); biases.append(padb(b2[i]))
    weights.append(padw(w_out)); biases.append(padb(b_out))
    return weights, biases


# ---------------------------------------------------------------------------
# Main entry
# ---------------------------------------------------------------------------
TRACE = False
LAST_RESULT = None


def prepare(inputs, edges, w_in, b_in, w1, b1, w2, b2, w_out, b_out):
    """Build plan + per-core inputs + compiled Bacc graph."""
    import concourse.bacc as bacc
    import concourse.tile as tile
    import concourse.mybir as mybir

    inputs = np.asarray(inputs, np.float32)
    edges_np = np.asarray(edges, np.int64)
    n_blocks = np.asarray(w1).shape[0]
    n_layers = 2 * n_blocks + 2

    plan = build_plan(edges_np, inputs.shape[0])
    _fix_idx_counts(plan)
    weights, biases = pack_weights(
        np.asarray(w_in), np.asarray(b_in), np.asarray(w1), np.asarray(b1),
        np.asarray(w2), np.asarray(b2), np.asarray(w_out), np.asarray(b_out),
        n_blocks)
    in_maps = build_in_maps(plan, inputs, weights, biases)

    nc = bacc.Bacc("TRN2", target_bir_lowering=False, debug=False,
                   num_devices=N_CORES)
    io = {}
    for name, arr in in_maps[0].items():
        io[name] = nc.dram_tensor(name, list(arr.shape),
                                  mybir.dt.from_np(arr.dtype),
                                  kind="ExternalInput").ap()
    for name in ("x_t", "x_out_t"):
        io[name] = nc.dram_tensor(name, [P, plan["s_pad"]],
                                  mybir.dt.float32,
                                  kind="ExternalOutput").ap()
    with tile.TileContext(nc) as tc:
        emit_kernel(tc, io, plan, n_layers)
    nc.compile()
    return nc, in_maps, plan


def kernel(inputs, edges, w_in, b_in, w1, b1, w2, b2, w_out, b_out):
    from concourse.bass_utils import run_bass_kernel_spmd

    nc, in_maps, plan = prepare(inputs, edges, w_in, b_in, w1, b1, w2, b2,
                                w_out, b_out)
    global LAST_RESULT
    res = run_bass_kernel_spmd(nc, in_maps, core_ids=list(range(N_CORES)),
                               trace=TRACE)
    LAST_RESULT = res
    x_out, x = unshard_outputs(plan, res.results)
    return x_out, x


if __name__ == "__main__":
    pass
